# revision 1
# baseline (speedup 1.0000x reference)
"""Trainium2 Bass kernel for NT-Xent contrastive loss (BATCH=4096, DIM=512, TEMP=0.5).

Strategy (data-parallel over rows of the 2B x 2B similarity matrix):
  - Host: E = concat(emb_i, emb_j) [8192, 512] f32, cast bf16. Each core gets
    ET = E.T (replicated) + its own 1024-column block, plus row-major copies
    (full + own + partner) pre-tiled into the SBUF image layout, an identity
    and a row-selector constant.
  - Device (per core, SPMD, no collectives):
      * sumsq of every row via DVE scalar_tensor_tensor square+accumulate
      * r = 1/||e|| = exp(-0.5*ln(sumsq)) on ACT -- Exp and Ln share one
        activation table set, so the kernel never swaps tables
      * broadcast r across partitions with PE: transpose r-block via the
        tensor engine, then one selector matmul per row-tile
      * normalize the rhs copy column-wise in place: z_j = e_j * r_j (DVE)
      * S' = e_block^T @ Z on PE (bf16, fp32 accum); first two column groups
        as [128,1024] PSUM groups (early start), rest as [128,2048] pairs
      * ACT: exp(S' * r_row/TEMP) with fused row-sum accumulation
      * positives via DVE row-dots of own x partner row-major blocks
      * per-core partial: sum_rows(log(den - e^{1/TEMP}) - pos/TEMP) -> [1,1]
  - Host: loss = sum(partials) / (2B).

Emission order is deliberate: per-engine queue order paces the normalization
pipeline (DMA -> DVE sumsq -> ACT ln/exp -> PE broadcast -> DVE normalize)
just ahead of the PE/ACT main-loop stream.
"""

import math

import ml_dtypes
import numpy as np

BATCH = 4096
DIM = 512
TEMP = 0.5
B2 = 2 * BATCH              # 8192 rows/cols of the similarity matrix
NCORES = 8
RPC = B2 // NCORES          # 1024 rows per core
KT = DIM // 128             # 4 contraction chunks
CG = 8                      # column groups
CGW = B2 // CG              # 1024 columns per group
T8 = RPC // 128             # 8 row-tiles per group / per core
NBF = CGW // 512            # 512-wide matmuls per group
NG = 5                      # main groups per row-tile: c0, c1, cp1, cp2, cp3
EXP_DIAG = math.exp(1.0 / TEMP)

_CACHE = {}


def _build():
    import concourse.bacc as bacc
    import concourse.mybir as mybir
    import concourse.tile as tile

    f32 = mybir.dt.float32
    bf16 = mybir.dt.bfloat16
    AF = mybir.ActivationFunctionType
    ALU = mybir.AluOpType
    X = mybir.AxisListType.X

    import bass_rust as _bass_rust
    from concourse.hw_specs import get_activation_tables

    class _Bacc(bacc.Bacc):
        """Bacc that pins Exp+Ln to the combined natural_log_exp_and_others
        activation-table set, so the kernel never swaps ACT tables."""

        def insert_act_table_loads(self):
            has_activation = any(
                isinstance(i, mybir.InstActivation)
                for b in self.main_func.blocks
                for i in b.instructions)
            if not has_activation:
                return
            drop = {mybir.ActivationFunctionType.Exp,
                    mybir.ActivationFunctionType.Ln}
            tables = []
            for name, funcs in get_activation_tables(self.m.arch).items():
                if name != "natural_log_exp_and_others":
                    funcs = funcs - drop
                tables.append((name, funcs))
            _bass_rust.insert_act_table_loads(self, tables)

    nc = _Bacc("TRN2", target_bir_lowering=False, debug=False,
               num_devices=NCORES)

    et_d = nc.dram_tensor("et", [DIM, B2], bf16, kind="ExternalInput").ap()
    etb_d = nc.dram_tensor("etb", [DIM, RPC], bf16, kind="ExternalInput").ap()
    erm_d = nc.dram_tensor("erm", [128, (B2 // 128) * DIM], bf16,
                           kind="ExternalInput").ap()
    ermb_d = nc.dram_tensor("ermb", [128, T8 * DIM], bf16,
                            kind="ExternalInput").ap()
    ermp_d = nc.dram_tensor("ermp", [128, T8 * DIM], bf16,
                            kind="ExternalInput").ap()
    iden_d = nc.dram_tensor("iden", [128, 128], bf16, kind="ExternalInput").ap()
    sel_d = nc.dram_tensor("sel", [128, T8 * 128], bf16,
                           kind="ExternalInput").ap()
    out_d = nc.dram_tensor("out", [1, 1], f32, kind="ExternalOutput").ap()

    with tile.TileContext(nc) as tc:
        with (
            tc.tile_pool(name="persist", bufs=1) as P,
            tc.tile_pool(name="scratch", bufs=2) as S,
            tc.tile_pool(name="psum", bufs=2, space="PSUM") as PS,
        ):
            ss64 = P.tile([128, 64], f32, name="ss64")
            ssb = P.tile([128, T8], f32, name="ssb")
            ssp = P.tile([128, T8], f32, name="ssp")
            rawpos = P.tile([128, T8], f32, name="rawpos")
            rsums = P.tile([128, T8 * NG], f32, name="rsums")
            sc8 = P.tile([128, T8], f32, name="sc8")
            pos8 = P.tile([128, T8], f32, name="pos8")
            ones = P.tile([128, 1], f32, name="ones")
            iden = P.tile([128, 128], bf16, name="iden")
            sel = P.tile([128, T8 * 128], bf16, name="sel")
            rbc = [P.tile([128, CGW], bf16, name=f"rbc_{c}") for c in range(CG)]
            erm = [None] * CG
            et2 = [[None] * CG for _ in range(KT)]   # raw (recycled scratch)
            etn = [[P.tile([128, CGW], bf16, name=f"etn_{k}_{c}")
                    for c in range(CG)] for k in range(KT)]
            etb = [None] * KT

            nc.vector.memset(ones[:], 1.0)
            nc.sync.dma_start(iden[:], iden_d[:])
            nc.sync.dma_start(sel[:], sel_d[:])

            def load_rm(dram_ap, name):
                sb = P.tile([128, T8 * DIM], bf16, name=name)
                nc.sync.dma_start(sb[:], dram_ap)
                return sb

            def load_erm(c):
                erm[c] = load_rm(erm_d[:, c * T8 * DIM:(c + 1) * T8 * DIM],
                                 f"erm_{c}")

            def load_et(c):
                for k in range(KT):
                    et2[k][c] = S.tile([128, CGW], bf16, name=f"et_{k}_{c}",
                                       tag="etraw", bufs=8)
                    nc.sync.dma_start(
                        et2[k][c][:],
                        et_d[k * 128:(k + 1) * 128, c * CGW:(c + 1) * CGW])

            def sumsq(src, tt, dst, dcol, src2=None):
                sco = S.tile([128, DIM], bf16, tag="stt", name="sco")
                s2 = src2 if src2 is not None else src
                nc.vector.scalar_tensor_tensor(
                    sco[:], src[:, tt * DIM:(tt + 1) * DIM], 1.0,
                    s2[:, tt * DIM:(tt + 1) * DIM], ALU.mult, ALU.mult,
                    accum_out=dst[:, dcol:dcol + 1])

            def rsqrt(dst, src_ap, w):
                """dst[:, 0:w] = 1/sqrt(src) via exp(-0.5*ln(x)) -- same ACT
                table set as the main-loop Exp, so no table swaps."""
                ln = S.tile([128, w], f32, tag=f"ln{w}", name="ln")
                nc.scalar.activation(ln[:], src_ap, AF.Ln)
                nc.scalar.activation(dst, ln[:], AF.Exp, scale=-0.5)

            def rchain(c):
                """r for group c -> broadcast via PE -> normalize in place."""
                rcb = S.tile([128, 128], bf16, tag="rcb", name="rcb")
                nc.vector.memset(rcb[:], 0.0)
                rsqrt(rcb[:, 0:T8], ss64[:, c * 8:(c + 1) * 8], T8)
                ptr = PS.tile([128, 128], bf16, tag="mm", name="ptr")
                nc.tensor.transpose(ptr[:], rcb[:], iden[:])
                rT = S.tile([128, 128], bf16, tag="rT", name="rT")
                nc.vector.tensor_copy(rT[:], ptr[:])
                pb = PS.tile([128, CGW], f32, tag="mm", name="pb")
                for t in range(T8):
                    nc.tensor.matmul(pb[:, t * 128:(t + 1) * 128],
                                     sel[:, t * 128:(t + 1) * 128],
                                     rT[:], start=True, stop=True)
                nc.vector.tensor_copy(rbc[c][:], pb[:])
                eng = nc.vector if c < 2 else nc.gpsimd
                for k in range(KT):
                    eng.tensor_tensor(etn[k][c][:], et2[k][c][:],
                                      rbc[c][:], ALU.mult)

            def main_group(gi, cgs):
                """One main group per row-tile over the given column groups."""
                for t in range(T8):
                    wid = len(cgs) * CGW
                    ps = PS.tile([128, wid], f32, tag="mm", name="psmm")
                    for k in range(KT):
                        for ci, c in enumerate(cgs):
                            for n in range(NBF):
                                lo = ci * CGW + n * 512
                                nc.tensor.matmul(
                                    ps[:, lo:lo + 512],
                                    etb[k][:, t * 128:(t + 1) * 128],
                                    etn[k][c][:, n * 512:(n + 1) * 512],
                                    start=(k == 0), stop=(k == KT - 1))
                    sce = S.tile([128, wid], bf16, tag="expout", name="sce")
                    col = t * NG + gi
                    nc.scalar.activation(sce[:], ps[:], AF.Exp,
                                         scale=sc8[:, t:t + 1],
                                         accum_out=rsums[:, col:col + 1])

            # ---- paced emission ----
            load_erm(0)
            ermb = load_rm(ermb_d[:, :], "ermb")
            load_erm(1)
            for k in range(KT):
                etb[k] = P.tile([128, RPC], bf16, name=f"etb_{k}")
                nc.sync.dma_start(etb[k][:], etb_d[k * 128:(k + 1) * 128, :])
            load_et(0)
            load_et(1)
            for tt in range(T8):
                sumsq(erm[0], tt, ss64, tt)
            rchain(0)
            for t in range(T8):                      # own norms
                sumsq(ermb, t, ssb, t)
            rb8 = P.tile([128, T8], f32, name="rb8")
            rsqrt(rb8[:], ssb[:], T8)
            nc.vector.tensor_scalar_mul(sc8[:], rb8[:], 1.0 / TEMP)
            main_group(0, (0,))

            load_erm(2)
            load_erm(3)
            load_et(2)
            load_et(3)
            for tt in range(T8):
                sumsq(erm[1], tt, ss64, 8 + tt)
            rchain(1)
            main_group(1, (1,))

            ermp = load_rm(ermp_d[:, :], "ermp")
            load_erm(4)
            load_erm(5)
            load_et(4)
            load_et(5)
            for c in (2, 3):
                for tt in range(T8):
                    sumsq(erm[c], tt, ss64, c * 8 + tt)
                rchain(c)
            main_group(2, (2, 3))

            load_erm(6)
            load_erm(7)
            load_et(6)
            load_et(7)
            for c in (4, 5):
                for tt in range(T8):
                    sumsq(erm[c], tt, ss64, c * 8 + tt)
                rchain(c)
            main_group(3, (4, 5))

            for c in (6, 7):
                for tt in range(T8):
                    sumsq(erm[c], tt, ss64, c * 8 + tt)
                rchain(c)
            for t in range(T8):                      # partner norms + positives
                sumsq(ermp, t, ssp, t)
            for t in range(T8):
                sumsq(ermb, t, rawpos, t, src2=ermp)
            rp8 = P.tile([128, T8], f32, name="rp8")
            rsqrt(rp8[:], ssp[:], T8)
            pt0 = P.tile([128, T8], f32, name="pt0")
            nc.vector.tensor_mul(pt0[:], rawpos[:], rb8[:])
            pt1 = P.tile([128, T8], f32, name="pt1")
            nc.vector.tensor_mul(pt1[:], pt0[:], rp8[:])
            nc.vector.tensor_scalar_mul(pos8[:], pt1[:], 1.0 / TEMP)

            main_group(4, (6, 7))

            # ---- finalize: den = rowsum - e^{1/T}; sum(log(den) - pos) ----
            den8 = P.tile([128, T8], f32, name="den8")
            nc.vector.tensor_reduce(
                den8[:], rsums[:].rearrange("p (t c) -> p t c", c=NG),
                X, ALU.add)
            den8b = P.tile([128, T8], f32, name="den8b")
            nc.vector.tensor_scalar_add(den8b[:], den8[:], -EXP_DIAG)
            logd = S.tile([128, T8], f32, tag="logd", name="logd")
            tlog = P.tile([128, 1], f32, name="tlog")
            nc.scalar.activation(logd[:], den8b[:], AF.Ln, accum_out=tlog[:])
            tpos = P.tile([128, 1], f32, name="tpos")
            nc.vector.tensor_reduce(tpos[:], pos8[:], X, ALU.add)
            lv = P.tile([128, 1], f32, name="lv")
            nc.vector.tensor_sub(lv[:], tlog[:], tpos[:])
            psf = PS.tile([1, 1], f32, tag="mm", name="psf")
            nc.tensor.matmul(psf[:], lv[:], ones[:], start=True, stop=True)
            ob = P.tile([1, 1], f32, name="ob")
            nc.vector.tensor_copy(ob[:], psf[:])
            nc.sync.dma_start(out_d[:], ob[:])

    nc.compile()
    return nc


def _get_nc():
    if "nc" not in _CACHE:
        _CACHE["nc"] = _build()
    return _CACHE["nc"]


def _in_maps(emb_i, emb_j):
    bf = ml_dtypes.bfloat16
    E = np.concatenate([np.asarray(emb_i, dtype=np.float32),
                        np.asarray(emb_j, dtype=np.float32)], axis=0)
    Ebf = E.astype(bf)                              # [8192, 512] row-major
    ET = np.ascontiguousarray(Ebf.T)                # [512, 8192]
    # SBUF-image tiling of the row-major copy: ERMT[p, t*512+d] = Ebf[t*128+p, d]
    ERMT = np.ascontiguousarray(
        Ebf.reshape(B2 // 128, 128, DIM).transpose(1, 0, 2).reshape(128, -1))
    SEL = np.zeros((128, T8 * 128), dtype=bf)
    for tp in range(T8):
        SEL[tp, tp * 128:(tp + 1) * 128] = 1.0
    maps = []
    for k in range(NCORES):
        s = k * RPC
        p = (s + BATCH) % B2
        maps.append({
            "et": ET,
            "etb": np.ascontiguousarray(ET[:, s:s + RPC]),
            "erm": ERMT,
            "ermb": np.ascontiguousarray(
                ERMT[:, s // 128 * DIM:(s // 128 + T8) * DIM]),
            "ermp": np.ascontiguousarray(
                ERMT[:, p // 128 * DIM:(p // 128 + T8) * DIM]),
            "iden": np.eye(128, dtype=bf),
            "sel": SEL,
        })
    return maps


def _run(emb_i, emb_j, trace=False):
    from concourse.bass_utils import run_bass_kernel_spmd
    nc = _get_nc()
    res = run_bass_kernel_spmd(nc, _in_maps(emb_i, emb_j),
                               list(range(NCORES)), trace=trace)
    total = sum(float(res.results[i]["out"][0, 0]) for i in range(NCORES))
    loss = np.float32(total / B2)
    return loss, res


def kernel(emb_i, emb_j):
    return _run(emb_i, emb_j, trace=False)[0]



# revision 2
# speedup vs baseline: 2.6389x; 2.6389x over previous
"""Trainium2 Bass kernel for NT-Xent contrastive loss (BATCH=4096, DIM=512, TEMP=0.5).

v2 strategy — exploit the symmetry of the similarity matrix + fp8 DoubleRow:
  - Host: L2-normalize rows of E = concat(emb_i, emb_j) in f32, compute the
    positive-pair dots and the (quantized) diagonal terms exactly, then cast
    z*16 to TRN fp8-e4m3 for the big matmul.
  - The 8192x8192 exp(sim/T) row-sum is split by symmetry: the 64x64 grid of
    128x128 tiles is covered by giving each block-row r the cyclic strip of
    tiles (r, r+c mod 64) for c = 0..32.  Tiles c = 1..31 contribute their
    row-sums to block r's denominators AND their column-sums (via symmetry
    s_ij = s_ji) to the denominators of rows in block r+c.  Tile c = 0 is the
    in-block tile (row-sums only); tile c = 32 appears in both orderings'
    strips, so it is row-sum only as well.  Every ordered pair (i, j) is
    covered exactly once; the self term exp(s_ii/T) is subtracted on host.
  - Core k owns block-rows 8k..8k+7; its rhs is the 5120-column cyclic window
    of z^T starting at column 1024k, so every core runs the IDENTICAL program
    on its own data (SPMD, no collectives).
  - Device per strip: fp8 DoubleRow matmuls (contraction 512 = 2 pairs of 128
    partitions x 2) into [128, 2048] PSUM chunks -> ACT exp (scale = 2/256)
    into a bf16 strip buffer -> DVE row-sum reduce.  Column-sums run as a PE
    tail: all-ones [128,128] weights x exp-strip slices, accumulated across
    strips in shared PSUM (three column phases), DVE-copied and DMA'd out.
  - Host: den = rowsum + colsum - diag; loss = mean(log(den) - pos/TEMP).
"""

import math

import ml_dtypes
import numpy as np

BATCH = 4096
DIM = 512
TEMP = 0.5
B2 = 2 * BATCH            # 8192 rows of the similarity matrix
NCORES = 8
NBLK = B2 // 128          # 64 block-rows
SPB = NBLK // NCORES      # 8 strips (block-rows) per core
NT = 33                   # tiles per strip (c = 0..32)
SW = NT * 128             # 4224 strip width (stream columns per strip)
TOTAL = SPB * SW          # 33792 stream columns per core
LCOLS = 128 * (SPB - 1) + SW   # 5120 local rhs columns per core
CH = 2048                 # main chunk width (PSUM double buffer)
SCALE = 16.0              # fp8 pre-scale on z
ACT_SCALE = (1.0 / TEMP) / (SCALE * SCALE)   # exp(s_hat * ACT_SCALE)
CS_LO = 128               # colsum window (local cols), strips j: [128j+128, 128j+4096)
CS_HI = 128 * (SPB - 1) + 4096   # 4992
CSW = CS_HI - CS_LO       # 4864
PHASES = [(128, 1792), (1792, 3456), (3456, 4992)]

_CACHE = {}


def _build():
    import concourse.bacc as bacc
    import concourse.mybir as mybir
    import concourse.tile as tile

    f32 = mybir.dt.float32
    bf16 = mybir.dt.bfloat16
    fp8 = mybir.dt.float8e4
    AF = mybir.ActivationFunctionType
    ALU = mybir.AluOpType
    X = mybir.AxisListType.X
    DR = mybir.MatmulPerfMode.DoubleRow

    nc = bacc.Bacc("TRN2", target_bir_lowering=False, debug=False,
                   num_devices=NCORES)

    wq_d = nc.dram_tensor("wq", [128, SPB * 512], fp8, kind="ExternalInput").ap()
    xq_d = nc.dram_tensor("xq", [128, 4 * LCOLS], fp8, kind="ExternalInput").ap()
    rowout_d = nc.dram_tensor("rowout", [128, SPB], f32,
                              kind="ExternalOutput").ap()
    colout_d = nc.dram_tensor("colout", [1, CSW], f32,
                              kind="ExternalOutput").ap()

    with tile.TileContext(nc) as tc:
        with (
            tc.tile_pool(name="persist", bufs=1) as P,
            tc.tile_pool(name="psum", bufs=2, space="PSUM") as PS,
        ):
            ones = P.tile([128, 128], bf16, name="ones")
            wq = P.tile([128, SPB * 512], fp8, name="wq")
            xq = P.tile([128, 4 * LCOLS], fp8, name="xq")
            exps = P.tile([128, TOTAL], bf16, name="exps")
            rowacc = P.tile([128, SPB], f32, name="rowacc")
            colsb = P.tile([1, CSW], f32, name="colsb")

            nc.vector.memset(ones[:], 1.0)
            # weights for the first strips, then the first rhs half (all 4
            # contraction subtiles), then the rest.
            for j in range(4):
                nc.sync.dma_start(wq[:, 512 * j:512 * (j + 1)],
                                  wq_d[:, 512 * j:512 * (j + 1)])
            for s in range(4):
                sl = slice(LCOLS * s, LCOLS * s + 2560)
                nc.sync.dma_start(xq[:, sl], xq_d[:, sl])
            for j in range(4, SPB):
                nc.sync.dma_start(wq[:, 512 * j:512 * (j + 1)],
                                  wq_d[:, 512 * j:512 * (j + 1)])
            for s in range(4):
                sl = slice(LCOLS * s + 2560, LCOLS * s + LCOLS)
                nc.sync.dma_start(xq[:, sl], xq_d[:, sl])

            xq3 = xq[:].rearrange("p (s c) -> p s c", s=4)
            wj3 = [wq[:, 512 * j:512 * (j + 1)].rearrange(
                "p (s m) -> p s m", s=4) for j in range(SPB)]

            # stream pieces: cut at the 512 grid (PSUM zero regions) and at
            # strip boundaries
            cuts = sorted(set(
                [512 * m for m in range(TOTAL // 512 + 1)] +
                [SW * j for j in range(SPB + 1)]))
            pieces = list(zip(cuts, cuts[1:]))

            nchunks = (TOTAL + CH - 1) // CH
            reduced = set()
            for ci in range(nchunks):
                c0, c1 = CH * ci, min(TOTAL, CH * (ci + 1))
                cp = [pc for pc in pieces if pc[0] >= c0 and pc[1] <= c1]
                ps = PS.tile([128, c1 - c0], f32, tag="mm", name="ps")
                emit = []
                for j in sorted(set(a // SW for a, b in cp)):
                    for kk in range(2):
                        for (a, b) in cp:
                            if a // SW == j:
                                emit.append((j, kk, a, b))
                regions = {}
                for idx, (j, kk, a, b) in enumerate(emit):
                    regions.setdefault(a // 512, []).append(idx)
                starts = {v[0] for v in regions.values()}
                stops = {v[-1] for v in regions.values()}
                for idx, (j, kk, a, b) in enumerate(emit):
                    lc = a - 4096 * j
                    nc.tensor.matmul(
                        ps[:, a - c0:b - c0],
                        wj3[j][:, 2 * kk:2 * kk + 2, :],
                        xq3[:, 2 * kk:2 * kk + 2, lc:lc + (b - a)],
                        start=(idx in starts), stop=(idx in stops),
                        perf_mode=DR)
                nc.scalar.activation(exps[:, c0:c1], ps[:, 0:c1 - c0],
                                     AF.Exp, scale=ACT_SCALE)
                for j in range(SPB):
                    if j not in reduced and SW * (j + 1) <= c1:
                        nc.vector.tensor_reduce(
                            rowacc[:, j:j + 1], exps[:, SW * j:SW * (j + 1)],
                            X, ALU.add)
                        reduced.add(j)

            nc.sync.dma_start(rowout_d[:], rowacc[:])

            # column-sum tail: all-ones weights, accumulate across strips in
            # shared PSUM per column phase, then drain row 0.
            for (pa, pb) in PHASES:
                w = pb - pa
                cps = PS.tile([128, w], f32, tag="mm", name="cps")
                emit = []
                for j in range(SPB):
                    wa = max(pa, 128 * j + 128)
                    wb = min(pb, 128 * j + 4096)
                    if wa >= wb:
                        continue
                    grid = [pa + 512 * g for g in range(1, (w + 511) // 512 + 1)]
                    cpts = [wa] + [g for g in grid if wa < g < wb] + [wb]
                    for a, b in zip(cpts, cpts[1:]):
                        emit.append((j, a, b))
                regions = {}
                for idx, (j, a, b) in enumerate(emit):
                    regions.setdefault((a - pa) // 512, []).append(idx)
                starts = {v[0] for v in regions.values()}
                stops = {v[-1] for v in regions.values()}
                for idx, (j, a, b) in enumerate(emit):
                    so = 4096 * j + a
                    nc.tensor.matmul(
                        cps[:, a - pa:b - pa], ones[:],
                        exps[:, so:so + (b - a)],
                        start=(idx in starts), stop=(idx in stops))
                nc.vector.tensor_copy(colsb[0:1, pa - CS_LO:pb - CS_LO],
                                      cps[0:1, 0:w])
                nc.sync.dma_start(colout_d[0:1, pa - CS_LO:pb - CS_LO],
                                  colsb[0:1, pa - CS_LO:pb - CS_LO])

    nc.compile()
    return nc


def _get_nc():
    if "nc" not in _CACHE:
        _CACHE["nc"] = _build()
    return _CACHE["nc"]


def _prep(emb_i, emb_j):
    fp8 = ml_dtypes.float8_e4m3
    E = np.concatenate([np.asarray(emb_i, dtype=np.float32),
                        np.asarray(emb_j, dtype=np.float32)], axis=0)
    nrm = np.sqrt((E * E).sum(axis=1, keepdims=True))
    Z = E / np.maximum(nrm, 1e-12)                       # [8192, 512] f32
    pos = (Z[:BATCH] * Z[BATCH:]).sum(axis=1)
    posf = np.concatenate([pos, pos]) / TEMP             # [8192]
    Zq = (Z * SCALE).astype(fp8)                         # [8192, 512] fp8
    Zqf = Zq.astype(np.float32) / SCALE
    diag = np.exp((Zqf * Zqf).sum(axis=1) / TEMP)        # [8192]
    ZqT = np.ascontiguousarray(Zq.T)                     # [512, 8192]
    maps = []
    for k in range(NCORES):
        Wb = ZqT[:, 1024 * k:1024 * (k + 1)]             # [512, 1024]
        wq = np.ascontiguousarray(
            Wb.reshape(4, 128, SPB, 128).transpose(1, 2, 0, 3).reshape(128, -1))
        cols = (1024 * k + np.arange(LCOLS)) % B2
        Xc = ZqT[:, cols]                                # [512, 5120]
        xq = np.ascontiguousarray(
            Xc.reshape(4, 128, LCOLS).transpose(1, 0, 2).reshape(128, -1))
        maps.append({"wq": wq, "xq": xq})
    return maps, posf, diag


def _run(emb_i, emb_j, trace=False):
    from concourse.bass_utils import run_bass_kernel_spmd
    nc = _get_nc()
    maps, posf, diag = _prep(emb_i, emb_j)
    res = run_bass_kernel_spmd(nc, maps, list(range(NCORES)), trace=trace)
    den = np.zeros(B2, dtype=np.float64)
    for k in range(NCORES):
        rowout = np.asarray(res.results[k]["rowout"], dtype=np.float64)
        colout = np.asarray(res.results[k]["colout"], dtype=np.float64)[0]
        rows = 1024 * k + np.arange(1024)
        den[rows] += rowout.T.reshape(-1)                # [p, j] -> row 128j+p
        g = (1024 * k + CS_LO + np.arange(CSW)) % B2
        den[g] += colout
    den = den - diag
    loss = np.float32(np.mean(np.log(den) - posf))
    return loss, res


def kernel(emb_i, emb_j):
    return _run(emb_i, emb_j, trace=False)[0]


# revision 7
# speedup vs baseline: 2.6962x; 1.0217x over previous
"""Trainium2 Bass kernel for NT-Xent contrastive loss (BATCH=4096, DIM=512, TEMP=0.5).

v2 strategy — exploit the symmetry of the similarity matrix + fp8 DoubleRow:
  - Host: L2-normalize rows of E = concat(emb_i, emb_j) in f32, compute the
    positive-pair dots and the (quantized) diagonal terms exactly, then cast
    z*16 to TRN fp8-e4m3 for the big matmul.
  - The 8192x8192 exp(sim/T) row-sum is split by symmetry: the 64x64 grid of
    128x128 tiles is covered by giving each block-row r the cyclic strip of
    tiles (r, r+c mod 64) for c = 0..32.  Tiles c = 1..31 contribute their
    row-sums to block r's denominators AND their column-sums (via symmetry
    s_ij = s_ji) to the denominators of rows in block r+c.  Tile c = 0 is the
    in-block tile (row-sums only); tile c = 32 appears in both orderings'
    strips, so it is row-sum only as well.  Every ordered pair (i, j) is
    covered exactly once; the self term exp(s_ii/T) is subtracted on host.
  - Core k owns block-rows 8k..8k+7; its rhs is the 5120-column cyclic window
    of z^T starting at column 1024k, so every core runs the IDENTICAL program
    on its own data (SPMD, no collectives).
  - Device per strip: fp8 DoubleRow matmuls (contraction 512 = 2 pairs of 128
    partitions x 2) into [128, 2048] PSUM chunks -> ACT exp (scale = 2/256)
    into a bf16 strip buffer -> DVE row-sum reduce.  Column-sums run as a PE
    tail: all-ones [128,128] weights x exp-strip slices, accumulated across
    strips in shared PSUM (three column phases), DVE-copied and DMA'd out.
  - Host: den = rowsum + colsum - diag; loss = mean(log(den) - pos/TEMP).
"""

import math

import ml_dtypes
import numpy as np

BATCH = 4096
DIM = 512
TEMP = 0.5
B2 = 2 * BATCH            # 8192 rows of the similarity matrix
NCORES = 8
NBLK = B2 // 128          # 64 block-rows
SPB = NBLK // NCORES      # 8 strips (block-rows) per core
NT = 33                   # tiles per strip (c = 0..32)
SW = NT * 128             # 4224 strip width (stream columns per strip)
TOTAL = SPB * SW          # 33792 stream columns per core
LCOLS = 128 * (SPB - 1) + SW   # 5120 local rhs columns per core
CH = 2048                 # main chunk width (PSUM double buffer)
SCALE = 16.0              # fp8 pre-scale on z
ACT_SCALE = (1.0 / TEMP) / (SCALE * SCALE)   # exp(s_hat * ACT_SCALE)
CS_LO = 128               # colsum window (local cols), strips j: [128j+128, 128j+4096)
CS_HI = 128 * (SPB - 1) + 4096   # 4992
CSW = CS_HI - CS_LO       # 4864
PHASES = [(128, 1920), (1920, 3712), (3712, 4992)]
RS = 34                   # padded per-strip tile-sum columns (4B-aligned bf16)

_CACHE = {}


def _build():
    import concourse.bacc as bacc
    import concourse.mybir as mybir
    import concourse.tile as tile

    f32 = mybir.dt.float32
    bf16 = mybir.dt.bfloat16
    fp8 = mybir.dt.float8e4
    AF = mybir.ActivationFunctionType
    ALU = mybir.AluOpType
    X = mybir.AxisListType.X
    DR = mybir.MatmulPerfMode.DoubleRow

    nc = bacc.Bacc("TRN2", target_bir_lowering=False, debug=False,
                   num_devices=NCORES)

    wq_d = nc.dram_tensor("wq", [128, SPB * 512], fp8, kind="ExternalInput").ap()
    xq_d = nc.dram_tensor("xq", [128, 4 * LCOLS], fp8, kind="ExternalInput").ap()
    rowout_d = nc.dram_tensor("rowout", [128, SPB], f32,
                              kind="ExternalOutput").ap()
    colout_d = nc.dram_tensor("colout", [1, CSW], f32,
                              kind="ExternalOutput").ap()

    with tile.TileContext(nc) as tc:
        with (
            tc.tile_pool(name="persist", bufs=1) as P,
            tc.tile_pool(name="psum", bufs=2, space="PSUM") as PS,
        ):
            ones = P.tile([128, 128], bf16, name="ones")
            wq = P.tile([128, SPB * 512], fp8, name="wq")
            xq = P.tile([128, 4 * LCOLS], fp8, name="xq")
            exps = P.tile([128, TOTAL], bf16, name="exps")
            rsum1 = P.tile([128, SPB * RS], bf16, name="rsum1")
            rowacc = P.tile([128, SPB], f32, name="rowacc")
            colsb = P.tile([1, CSW], f32, name="colsb")

            nc.vector.memset(ones[:], 1.0)
            nc.vector.memset(rsum1[:], 0.0)
            # split input loads over the two queues: sync gets what the first
            # chunks need, gpsimd (software DGE) prefetches the rest.
            nc.sync.dma_start(wq[:, 0:2048], wq_d[:, 0:2048])
            for s in range(4):
                sl = slice(LCOLS * s, LCOLS * s + 2560)
                nc.sync.dma_start(xq[:, sl], xq_d[:, sl])
            for s in range(4):
                sl = slice(LCOLS * s + 2560, LCOLS * s + LCOLS)
                nc.gpsimd.dma_start(xq[:, sl], xq_d[:, sl])
            nc.gpsimd.dma_start(wq[:, 2048:4096], wq_d[:, 2048:4096])

            xq3 = xq[:].rearrange("p (s c) -> p s c", s=4)
            wj3 = [wq[:, 512 * j:512 * (j + 1)].rearrange(
                "p (s m) -> p s m", s=4) for j in range(SPB)]

            # stream pieces: cut at the 512 grid (PSUM zero regions) and at
            # strip boundaries
            cuts = sorted(set(
                [512 * m for m in range(TOTAL // 512 + 1)] +
                [SW * j for j in range(SPB + 1)]))
            pieces = list(zip(cuts, cuts[1:]))

            nchunks = (TOTAL + CH - 1) // CH
            reduced = set()
            for ci in range(nchunks):
                c0, c1 = CH * ci, min(TOTAL, CH * (ci + 1))
                cp = [pc for pc in pieces if pc[0] >= c0 and pc[1] <= c1]
                ps = PS.tile([128, c1 - c0], f32, tag="mm", name="ps")
                emit = []
                for j in sorted(set(a // SW for a, b in cp)):
                    for kk in range(2):
                        for (a, b) in cp:
                            if a // SW == j:
                                emit.append((j, kk, a, b))
                regions = {}
                for idx, (j, kk, a, b) in enumerate(emit):
                    regions.setdefault(a // 512, []).append(idx)
                starts = {v[0] for v in regions.values()}
                stops = {v[-1] for v in regions.values()}
                for idx, (j, kk, a, b) in enumerate(emit):
                    lc = a - 4096 * j
                    nc.tensor.matmul(
                        ps[:, a - c0:b - c0],
                        wj3[j][:, 2 * kk:2 * kk + 2, :],
                        xq3[:, 2 * kk:2 * kk + 2, lc:lc + (b - a)],
                        start=(idx in starts), stop=(idx in stops),
                        perf_mode=DR)
                nc.scalar.activation(exps[:, c0:c1], ps[:, 0:c1 - c0],
                                     AF.Exp, scale=ACT_SCALE)
                for j in range(SPB):
                    if j not in reduced and SW * (j + 1) <= c1:
                        # stage-1 row sums: per-tile partials, all-bf16 so the
                        # DVE 2X_1PORT mode engages (dst slice is 4B-aligned)
                        with nc.allow_low_precision(
                                reason="per-tile partials; stage-2 is f32"):
                            nc.vector.tensor_reduce(
                                rsum1[:, RS * j:RS * j + NT],
                                exps[:, SW * j:SW * (j + 1)].rearrange(
                                    "p (t c) -> p t c", c=128),
                                X, ALU.add)
                        reduced.add(j)

            # stage-2 row sums: tiny f32 reduce over the padded partials
            nc.vector.tensor_reduce(
                rowacc[:], rsum1[:].rearrange("p (j t) -> p j t", t=RS),
                X, ALU.add)
            nc.sync.dma_start(rowout_d[:], rowacc[:])

            # column-sum tail: all-ones weights, accumulate across strips in
            # shared PSUM per column phase, then drain row 0.
            for (pa, pb) in PHASES:
                w = pb - pa
                cps = PS.tile([128, w], f32, tag="mm", name="cps")
                emit = []
                for j in range(SPB):
                    wa = max(pa, 128 * j + 128)
                    wb = min(pb, 128 * j + 4096)
                    if wa >= wb:
                        continue
                    grid = [pa + 512 * g for g in range(1, (w + 511) // 512 + 1)]
                    cpts = [wa] + [g for g in grid if wa < g < wb] + [wb]
                    for a, b in zip(cpts, cpts[1:]):
                        emit.append((j, a, b))
                regions = {}
                for idx, (j, a, b) in enumerate(emit):
                    regions.setdefault((a - pa) // 512, []).append(idx)
                starts = {v[0] for v in regions.values()}
                stops = {v[-1] for v in regions.values()}
                for idx, (j, a, b) in enumerate(emit):
                    so = 4096 * j + a
                    nc.tensor.matmul(
                        cps[:, a - pa:b - pa], ones[:],
                        exps[:, so:so + (b - a)],
                        start=(idx in starts), stop=(idx in stops))
                nc.vector.tensor_copy(colsb[0:1, pa - CS_LO:pb - CS_LO],
                                      cps[0:1, 0:w])
                nc.sync.dma_start(colout_d[0:1, pa - CS_LO:pb - CS_LO],
                                  colsb[0:1, pa - CS_LO:pb - CS_LO])

    nc.compile()
    return nc


def _get_nc():
    if "nc" not in _CACHE:
        _CACHE["nc"] = _build()
    return _CACHE["nc"]


def _prep(emb_i, emb_j):
    fp8 = ml_dtypes.float8_e4m3
    E = np.concatenate([np.asarray(emb_i, dtype=np.float32),
                        np.asarray(emb_j, dtype=np.float32)], axis=0)
    nrm = np.sqrt((E * E).sum(axis=1, keepdims=True))
    Z = E / np.maximum(nrm, 1e-12)                       # [8192, 512] f32
    pos = (Z[:BATCH] * Z[BATCH:]).sum(axis=1)
    posf = np.concatenate([pos, pos]) / TEMP             # [8192]
    Zq = (Z * SCALE).astype(fp8)                         # [8192, 512] fp8
    Zqf = Zq.astype(np.float32) / SCALE
    diag = np.exp((Zqf * Zqf).sum(axis=1) / TEMP)        # [8192]
    ZqT = np.ascontiguousarray(Zq.T)                     # [512, 8192]
    maps = []
    for k in range(NCORES):
        Wb = ZqT[:, 1024 * k:1024 * (k + 1)]             # [512, 1024]
        wq = np.ascontiguousarray(
            Wb.reshape(4, 128, SPB, 128).transpose(1, 2, 0, 3).reshape(128, -1))
        cols = (1024 * k + np.arange(LCOLS)) % B2
        Xc = ZqT[:, cols]                                # [512, 5120]
        xq = np.ascontiguousarray(
            Xc.reshape(4, 128, LCOLS).transpose(1, 0, 2).reshape(128, -1))
        maps.append({"wq": wq, "xq": xq})
    return maps, posf, diag


def _run(emb_i, emb_j, trace=False):
    from concourse.bass_utils import run_bass_kernel_spmd
    nc = _get_nc()
    maps, posf, diag = _prep(emb_i, emb_j)
    res = run_bass_kernel_spmd(nc, maps, list(range(NCORES)), trace=trace)
    den = np.zeros(B2, dtype=np.float64)
    for k in range(NCORES):
        rowout = np.asarray(res.results[k]["rowout"], dtype=np.float64)
        colout = np.asarray(res.results[k]["colout"], dtype=np.float64)[0]
        rows = 1024 * k + np.arange(1024)
        den[rows] += rowout.T.reshape(-1)                # [p, j] -> row 128j+p
        g = (1024 * k + CS_LO + np.arange(CSW)) % B2
        den[g] += colout
    den = den - diag
    loss = np.float32(np.mean(np.log(den) - posf))
    return loss, res


def kernel(emb_i, emb_j):
    return _run(emb_i, emb_j, trace=False)[0]


# revision 10
# speedup vs baseline: 2.7383x; 1.0156x over previous
"""Trainium2 Bass kernel for NT-Xent contrastive loss (BATCH=4096, DIM=512, TEMP=0.5).

v2 strategy — exploit the symmetry of the similarity matrix + fp8 DoubleRow:
  - Host: L2-normalize rows of E = concat(emb_i, emb_j) in f32, compute the
    positive-pair dots and the (quantized) diagonal terms exactly, then cast
    z*16 to TRN fp8-e4m3 for the big matmul.
  - The 8192x8192 exp(sim/T) row-sum is split by symmetry: the 64x64 grid of
    128x128 tiles is covered by giving each block-row r the cyclic strip of
    tiles (r, r+c mod 64) for c = 0..32.  Tiles c = 1..31 contribute their
    row-sums to block r's denominators AND their column-sums (via symmetry
    s_ij = s_ji) to the denominators of rows in block r+c.  Tile c = 0 is the
    in-block tile (row-sums only); tile c = 32 appears in both orderings'
    strips, so it is row-sum only as well.  Every ordered pair (i, j) is
    covered exactly once; the self term exp(s_ii/T) is subtracted on host.
  - Core k owns block-rows 8k..8k+7; its rhs is the 5120-column cyclic window
    of z^T starting at column 1024k, so every core runs the IDENTICAL program
    on its own data (SPMD, no collectives).
  - Device per strip: fp8 DoubleRow matmuls (contraction 512 = 2 pairs of 128
    partitions x 2) into [128, 2048] PSUM chunks -> ACT exp (scale = 2/256)
    into a bf16 strip buffer -> DVE row-sum reduce.  Column-sums run as a PE
    tail: all-ones [128,128] weights x exp-strip slices, accumulated across
    strips in shared PSUM (three column phases), DVE-copied and DMA'd out.
  - Host: den = rowsum + colsum - diag; loss = mean(log(den) - pos/TEMP).
"""

import math

import ml_dtypes
import numpy as np

BATCH = 4096
DIM = 512
TEMP = 0.5
B2 = 2 * BATCH            # 8192 rows of the similarity matrix
NCORES = 8
NBLK = B2 // 128          # 64 block-rows
SPB = NBLK // NCORES      # 8 strips (block-rows) per core
NT = 33                   # tiles per strip (c = 0..32)
SW = NT * 128             # 4224 strip width (stream columns per strip)
TOTAL = SPB * SW          # 33792 stream columns per core
LCOLS = 128 * (SPB - 1) + SW   # 5120 local rhs columns per core
CH = 2048                 # main chunk width (PSUM double buffer)
SCALE = 16.0              # fp8 pre-scale on z
ACT_SCALE = (1.0 / TEMP) / (SCALE * SCALE)   # exp(s_hat * ACT_SCALE)
CS_LO = 128               # colsum window (local cols), strips j: [128j+128, 128j+4096)
CS_HI = 128 * (SPB - 1) + 4096   # 4992
CSW = CS_HI - CS_LO       # 4864
PHASES = [(128, 1920), (1920, 3712), (3712, 4992)]

_CACHE = {}


def _build():
    import concourse.bacc as bacc
    import concourse.mybir as mybir
    import concourse.tile as tile

    f32 = mybir.dt.float32
    bf16 = mybir.dt.bfloat16
    fp8 = mybir.dt.float8e4
    AF = mybir.ActivationFunctionType
    ALU = mybir.AluOpType
    X = mybir.AxisListType.X
    DR = mybir.MatmulPerfMode.DoubleRow

    nc = bacc.Bacc("TRN2", target_bir_lowering=False, debug=False,
                   num_devices=NCORES)

    wq_d = nc.dram_tensor("wq", [128, SPB * 512], fp8, kind="ExternalInput").ap()
    xq_d = nc.dram_tensor("xq", [128, 4 * LCOLS], fp8, kind="ExternalInput").ap()
    rowout_d = nc.dram_tensor("rowout", [128, SPB], f32,
                              kind="ExternalOutput").ap()
    colout_d = nc.dram_tensor("colout", [1, CSW], f32,
                              kind="ExternalOutput").ap()

    with tile.TileContext(nc) as tc:
        with (
            tc.tile_pool(name="persist", bufs=1) as P,
            tc.tile_pool(name="scratch", bufs=2) as S,
            tc.tile_pool(name="psum", bufs=2, space="PSUM") as PS,
        ):
            ones = P.tile([128, 128], bf16, name="ones")
            wq = P.tile([128, SPB * 512], fp8, name="wq")
            xq = P.tile([128, 4 * LCOLS], fp8, name="xq")
            exps = P.tile([128, TOTAL], bf16, name="exps")
            rowacc = P.tile([128, SPB], f32, name="rowacc")
            colsb = P.tile([1, CSW], f32, name="colsb")

            nc.vector.memset(ones[:], 1.0)
            # HBM loads: wq first (it doubles as xq cols [0,1024) via local
            # SBUF gathers), then the rest of xq split over the two queues.
            nc.sync.dma_start(wq[:], wq_d[:])
            for s in range(4):
                # xq[p, 5120s + 128j + m] = wq[p, 512j + 128s + m]
                src = wq[:].rearrange("p (j sm) -> p j sm", j=SPB)[
                    :, :, 128 * s:128 * (s + 1)]
                dst = xq[:, LCOLS * s:LCOLS * s + 1024].rearrange(
                    "p (j m) -> p j m", m=128)
                nc.sync.dma_start(dst, src)
            for s in range(4):
                sl = slice(LCOLS * s + 1024, LCOLS * s + 2560)
                nc.sync.dma_start(xq[:, sl], xq_d[:, sl])
            for s in range(4):
                sl = slice(LCOLS * s + 2560, LCOLS * s + LCOLS)
                nc.gpsimd.dma_start(xq[:, sl], xq_d[:, sl])

            xq3 = xq[:].rearrange("p (s c) -> p s c", s=4)
            wj3 = [wq[:, 512 * j:512 * (j + 1)].rearrange(
                "p (s m) -> p s m", s=4) for j in range(SPB)]

            # stream pieces: cut at the 512 grid (PSUM zero regions) and at
            # strip boundaries
            cuts = sorted(set(
                [512 * m for m in range(TOTAL // 512 + 1)] +
                [SW * j for j in range(SPB + 1)]))
            pieces = list(zip(cuts, cuts[1:]))

            nchunks = (TOTAL + CH - 1) // CH
            reduced = set()
            for ci in range(nchunks):
                c0, c1 = CH * ci, min(TOTAL, CH * (ci + 1))
                cp = [pc for pc in pieces if pc[0] >= c0 and pc[1] <= c1]
                ps = PS.tile([128, c1 - c0], f32, tag="mm", name="ps")
                emit = []
                for j in sorted(set(a // SW for a, b in cp)):
                    for kk in range(2):
                        for (a, b) in cp:
                            if a // SW == j:
                                emit.append((j, kk, a, b))
                regions = {}
                for idx, (j, kk, a, b) in enumerate(emit):
                    regions.setdefault(a // 512, []).append(idx)
                starts = {v[0] for v in regions.values()}
                stops = {v[-1] for v in regions.values()}
                for idx, (j, kk, a, b) in enumerate(emit):
                    lc = a - 4096 * j
                    nc.tensor.matmul(
                        ps[:, a - c0:b - c0],
                        wj3[j][:, 2 * kk:2 * kk + 2, :],
                        xq3[:, 2 * kk:2 * kk + 2, lc:lc + (b - a)],
                        start=(idx in starts), stop=(idx in stops),
                        perf_mode=DR)
                nc.scalar.activation(exps[:, c0:c1], ps[:, 0:c1 - c0],
                                     AF.Exp, scale=ACT_SCALE)
                for j in range(SPB):
                    if j not in reduced and SW * (j + 1) <= c1:
                        # row sums: bf16 pairwise tree folds (DVE 2X_1PORT
                        # needs all-2B operands), then a small f32 reduce
                        with nc.allow_low_precision(
                                reason="pairwise bf16 folds; final add is f32"):
                            a = exps[:, SW * j:SW * (j + 1)]
                            f1 = S.tile([128, 2112], bf16, tag="f1", name="f1")
                            nc.vector.tensor_tensor(
                                f1[:], a[:, 0:2112], a[:, 2112:4224], ALU.add)
                            f2 = S.tile([128, 1056], bf16, tag="f2", name="f2")
                            nc.vector.tensor_tensor(
                                f2[:], f1[:, 0:1056], f1[:, 1056:2112], ALU.add)
                            f3 = S.tile([128, 528], bf16, tag="f3", name="f3")
                            nc.vector.tensor_tensor(
                                f3[:], f2[:, 0:528], f2[:, 528:1056], ALU.add)
                        nc.vector.tensor_reduce(
                            rowacc[:, j:j + 1], f3[:], X, ALU.add)
                        reduced.add(j)

            nc.sync.dma_start(rowout_d[:], rowacc[:])

            # column-sum tail: all-ones weights, accumulate across strips in
            # shared PSUM per column phase, then drain row 0.
            for (pa, pb) in PHASES:
                w = pb - pa
                cps = PS.tile([128, w], f32, tag="mm", name="cps")
                emit = []
                for j in range(SPB):
                    wa = max(pa, 128 * j + 128)
                    wb = min(pb, 128 * j + 4096)
                    if wa >= wb:
                        continue
                    grid = [pa + 512 * g for g in range(1, (w + 511) // 512 + 1)]
                    cpts = [wa] + [g for g in grid if wa < g < wb] + [wb]
                    for a, b in zip(cpts, cpts[1:]):
                        emit.append((j, a, b))
                regions = {}
                for idx, (j, a, b) in enumerate(emit):
                    regions.setdefault((a - pa) // 512, []).append(idx)
                starts = {v[0] for v in regions.values()}
                stops = {v[-1] for v in regions.values()}
                for idx, (j, a, b) in enumerate(emit):
                    so = 4096 * j + a
                    nc.tensor.matmul(
                        cps[:, a - pa:b - pa], ones[:],
                        exps[:, so:so + (b - a)],
                        start=(idx in starts), stop=(idx in stops))
                nc.vector.tensor_copy(colsb[0:1, pa - CS_LO:pb - CS_LO],
                                      cps[0:1, 0:w])
                nc.sync.dma_start(colout_d[0:1, pa - CS_LO:pb - CS_LO],
                                  colsb[0:1, pa - CS_LO:pb - CS_LO])

    nc.compile()
    return nc


def _get_nc():
    if "nc" not in _CACHE:
        _CACHE["nc"] = _build()
    return _CACHE["nc"]


def _prep(emb_i, emb_j):
    fp8 = ml_dtypes.float8_e4m3
    E = np.concatenate([np.asarray(emb_i, dtype=np.float32),
                        np.asarray(emb_j, dtype=np.float32)], axis=0)
    nrm = np.sqrt((E * E).sum(axis=1, keepdims=True))
    Z = E / np.maximum(nrm, 1e-12)                       # [8192, 512] f32
    pos = (Z[:BATCH] * Z[BATCH:]).sum(axis=1)
    posf = np.concatenate([pos, pos]) / TEMP             # [8192]
    Zq = (Z * SCALE).astype(fp8)                         # [8192, 512] fp8
    Zqf = Zq.astype(np.float32) / SCALE
    diag = np.exp((Zqf * Zqf).sum(axis=1) / TEMP)        # [8192]
    ZqT = np.ascontiguousarray(Zq.T)                     # [512, 8192]
    maps = []
    for k in range(NCORES):
        Wb = ZqT[:, 1024 * k:1024 * (k + 1)]             # [512, 1024]
        wq = np.ascontiguousarray(
            Wb.reshape(4, 128, SPB, 128).transpose(1, 2, 0, 3).reshape(128, -1))
        cols = (1024 * k + np.arange(LCOLS)) % B2
        Xc = ZqT[:, cols]                                # [512, 5120]
        xq = np.ascontiguousarray(
            Xc.reshape(4, 128, LCOLS).transpose(1, 0, 2).reshape(128, -1))
        maps.append({"wq": wq, "xq": xq})
    return maps, posf, diag


def _run(emb_i, emb_j, trace=False):
    from concourse.bass_utils import run_bass_kernel_spmd
    nc = _get_nc()
    maps, posf, diag = _prep(emb_i, emb_j)
    res = run_bass_kernel_spmd(nc, maps, list(range(NCORES)), trace=trace)
    den = np.zeros(B2, dtype=np.float64)
    for k in range(NCORES):
        rowout = np.asarray(res.results[k]["rowout"], dtype=np.float64)
        colout = np.asarray(res.results[k]["colout"], dtype=np.float64)[0]
        rows = 1024 * k + np.arange(1024)
        den[rows] += rowout.T.reshape(-1)                # [p, j] -> row 128j+p
        g = (1024 * k + CS_LO + np.arange(CSW)) % B2
        den[g] += colout
    den = den - diag
    loss = np.float32(np.mean(np.log(den) - posf))
    return loss, res


def kernel(emb_i, emb_j):
    return _run(emb_i, emb_j, trace=False)[0]


# revision 15
# speedup vs baseline: 2.8745x; 1.0497x over previous
"""Trainium2 Bass kernel for NT-Xent contrastive loss (BATCH=4096, DIM=512, TEMP=0.5).

v2 strategy — exploit the symmetry of the similarity matrix + fp8 DoubleRow:
  - Host: L2-normalize rows of E = concat(emb_i, emb_j) in f32, compute the
    positive-pair dots and the (quantized) diagonal terms exactly, then cast
    z*16 to TRN fp8-e4m3 for the big matmul.
  - The 8192x8192 exp(sim/T) row-sum is split by symmetry: the 64x64 grid of
    128x128 tiles is covered by giving each block-row r the cyclic strip of
    tiles (r, r+c mod 64) for c = 0..32.  Tiles c = 1..31 contribute their
    row-sums to block r's denominators AND their column-sums (via symmetry
    s_ij = s_ji) to the denominators of rows in block r+c.  Tile c = 0 is the
    in-block tile (row-sums only); tile c = 32 appears in both orderings'
    strips, so it is row-sum only as well.  Every ordered pair (i, j) is
    covered exactly once; the self term exp(s_ii/T) is subtracted on host.
  - Core k owns block-rows 8k..8k+7; its rhs is the 5120-column cyclic window
    of z^T starting at column 1024k, so every core runs the IDENTICAL program
    on its own data (SPMD, no collectives).
  - Device per strip: fp8 DoubleRow matmuls (contraction 512 = 2 pairs of 128
    partitions x 2) into [128, 2048] PSUM chunks -> ACT exp (scale = 2/256)
    into a bf16 strip buffer -> DVE row-sum reduce.  Column-sums run as a PE
    tail: all-ones [128,128] weights x exp-strip slices, accumulated across
    strips in shared PSUM (three column phases), DVE-copied and DMA'd out.
  - Host: den = rowsum + colsum - diag; loss = mean(log(den) - pos/TEMP).
"""

import math

import ml_dtypes
import numpy as np

BATCH = 4096
DIM = 512
TEMP = 0.5
B2 = 2 * BATCH            # 8192 rows of the similarity matrix
NCORES = 8
NBLK = B2 // 128          # 64 block-rows
SPB = NBLK // NCORES      # 8 strips (block-rows) per core
NT = 33                   # tiles per strip (c = 0..32)
SW = NT * 128             # 4224 strip width (stream columns per strip)
TOTAL = SPB * SW          # 33792 stream columns per core
LCOLS = 128 * (SPB - 1) + SW   # 5120 local rhs columns per core
CH = 2048                 # main chunk width (PSUM double buffer)
SCALE = 16.0              # fp8 pre-scale on z
ACT_SCALE = (1.0 / TEMP) / (SCALE * SCALE)   # exp(s_hat * ACT_SCALE)
CS_LO = 128               # colsum window (local cols), strips j: [128j+128, 128j+4096)
CS_HI = 128 * (SPB - 1) + 4096   # 4992
CSW = CS_HI - CS_LO       # 4864
PHASES = [(128, 940), (940, 1752), (1752, 2564), (2564, 3376),
          (3376, 4188), (4188, 4992)]

_CACHE = {}


def _build():
    import concourse.bacc as bacc
    import concourse.mybir as mybir
    import concourse.tile as tile

    f32 = mybir.dt.float32
    bf16 = mybir.dt.bfloat16
    fp8 = mybir.dt.float8e4
    AF = mybir.ActivationFunctionType
    ALU = mybir.AluOpType
    X = mybir.AxisListType.X
    DR = mybir.MatmulPerfMode.DoubleRow

    nc = bacc.Bacc("TRN2", target_bir_lowering=False, debug=False,
                   num_devices=NCORES)

    xq_d = nc.dram_tensor("xq", [128, 4 * LCOLS], fp8, kind="ExternalInput").ap()
    rowout_d = nc.dram_tensor("rowout", [128, SPB], f32,
                              kind="ExternalOutput").ap()
    colout_d = nc.dram_tensor("colout", [1, CSW], f32,
                              kind="ExternalOutput").ap()

    with tile.TileContext(nc) as tc:
        with (
            tc.tile_pool(name="persist", bufs=1) as P,
            tc.tile_pool(name="scratch", bufs=2) as S,
            tc.tile_pool(name="psum", bufs=2, space="PSUM") as PS,
        ):
            ones = P.tile([128, 128], bf16, name="ones")
            xq = P.tile([128, 4 * LCOLS], fp8, name="xq")
            exps = P.tile([128, TOTAL], bf16, name="exps")
            rowacc = P.tile([128, SPB], f32, name="rowacc")
            colsb = P.tile([1, CSW], f32, name="colsb")

            nc.vector.memset(ones[:], 1.0)
            # HBM loads, first-needed first, balanced over the two DGE queues.
            # The matmul weights are read straight out of xq (cols [0,1024)
            # hold the core's own 1024 rows), so xq is the only input.
            for s in (0, 1):
                nc.sync.dma_start(xq[:, LCOLS * s:LCOLS * s + 2048],
                                  xq_d[:, LCOLS * s:LCOLS * s + 2048])
            for s in (2, 3):
                nc.gpsimd.dma_start(xq[:, LCOLS * s:LCOLS * s + 2048],
                                    xq_d[:, LCOLS * s:LCOLS * s + 2048])
            for s in (0, 1):
                sl = slice(LCOLS * s + 2048, LCOLS * s + LCOLS)
                nc.sync.dma_start(xq[:, sl], xq_d[:, sl])
            for s in (2, 3):
                sl = slice(LCOLS * s + 2048, LCOLS * s + LCOLS)
                nc.gpsimd.dma_start(xq[:, sl], xq_d[:, sl])

            xq3 = xq[:].rearrange("p (s c) -> p s c", s=4)

            # stream pieces: cut at the 512 grid (PSUM zero regions) and at
            # strip boundaries
            cuts = sorted(set(
                [512 * m for m in range(TOTAL // 512 + 1)] +
                [SW * j for j in range(SPB + 1)]))
            pieces = list(zip(cuts, cuts[1:]))

            nchunks = (TOTAL + CH - 1) // CH
            reduced = set()
            for ci in range(nchunks):
                c0, c1 = CH * ci, min(TOTAL, CH * (ci + 1))
                cp = [pc for pc in pieces if pc[0] >= c0 and pc[1] <= c1]
                ps = PS.tile([128, c1 - c0], f32, tag="mm", name="ps")
                emit = []
                for j in sorted(set(a // SW for a, b in cp)):
                    for kk in range(2):
                        for (a, b) in cp:
                            if a // SW == j:
                                emit.append((j, kk, a, b))
                regions = {}
                for idx, (j, kk, a, b) in enumerate(emit):
                    regions.setdefault(a // 512, []).append(idx)
                starts = {v[0] for v in regions.values()}
                stops = {v[-1] for v in regions.values()}
                for idx, (j, kk, a, b) in enumerate(emit):
                    lc = a - 4096 * j
                    nc.tensor.matmul(
                        ps[:, a - c0:b - c0],
                        xq3[:, 2 * kk:2 * kk + 2, 128 * j:128 * (j + 1)],
                        xq3[:, 2 * kk:2 * kk + 2, lc:lc + (b - a)],
                        start=(idx in starts), stop=(idx in stops),
                        perf_mode=DR)
                nc.scalar.activation(exps[:, c0:c1], ps[:, 0:c1 - c0],
                                     AF.Exp, scale=ACT_SCALE)
                for j in range(SPB):
                    if j not in reduced and SW * (j + 1) <= c1:
                        # row sums: bf16 pairwise tree folds (DVE 2X_1PORT
                        # needs all-2B operands), then a small f32 reduce
                        with nc.allow_low_precision(
                                reason="pairwise bf16 folds; final add is f32"):
                            a = exps[:, SW * j:SW * (j + 1)]
                            f1 = S.tile([128, 2112], bf16, tag="f1", name="f1")
                            nc.vector.tensor_tensor(
                                f1[:], a[:, 0:2112], a[:, 2112:4224], ALU.add)
                            f2 = S.tile([128, 1056], bf16, tag="f2", name="f2")
                            nc.vector.tensor_tensor(
                                f2[:], f1[:, 0:1056], f1[:, 1056:2112], ALU.add)
                            f3 = S.tile([128, 528], bf16, tag="f3", name="f3")
                            nc.vector.tensor_tensor(
                                f3[:], f2[:, 0:528], f2[:, 528:1056], ALU.add)
                        nc.vector.tensor_reduce(
                            rowacc[:, j:j + 1], f3[:], X, ALU.add)
                        reduced.add(j)

            nc.sync.dma_start(rowout_d[:], rowacc[:])

            # column-sum tail: all-ones weights, accumulate across strips in
            # shared PSUM per column phase, then drain row 0.
            for (pa, pb) in PHASES:
                w = pb - pa
                cps = PS.tile([128, w], f32, tag="mm", name="cps")
                emit = []
                for j in range(SPB):
                    wa = max(pa, 128 * j + 128)
                    wb = min(pb, 128 * j + 4096)
                    if wa >= wb:
                        continue
                    grid = [pa + 512 * g for g in range(1, (w + 511) // 512 + 1)]
                    cpts = [wa] + [g for g in grid if wa < g < wb] + [wb]
                    for a, b in zip(cpts, cpts[1:]):
                        emit.append((j, a, b))
                regions = {}
                for idx, (j, a, b) in enumerate(emit):
                    regions.setdefault((a - pa) // 512, []).append(idx)
                starts = {v[0] for v in regions.values()}
                stops = {v[-1] for v in regions.values()}
                for idx, (j, a, b) in enumerate(emit):
                    so = 4096 * j + a
                    nc.tensor.matmul(
                        cps[:, a - pa:b - pa], ones[:],
                        exps[:, so:so + (b - a)],
                        start=(idx in starts), stop=(idx in stops))
                nc.vector.tensor_copy(colsb[0:1, pa - CS_LO:pb - CS_LO],
                                      cps[0:1, 0:w])
                nc.sync.dma_start(colout_d[0:1, pa - CS_LO:pb - CS_LO],
                                  colsb[0:1, pa - CS_LO:pb - CS_LO])

    nc.compile()
    return nc


def _get_nc():
    if "nc" not in _CACHE:
        _CACHE["nc"] = _build()
    return _CACHE["nc"]


def _prep(emb_i, emb_j):
    fp8 = ml_dtypes.float8_e4m3
    E = np.concatenate([np.asarray(emb_i, dtype=np.float32),
                        np.asarray(emb_j, dtype=np.float32)], axis=0)
    nrm = np.sqrt((E * E).sum(axis=1, keepdims=True))
    Z = E / np.maximum(nrm, 1e-12)                       # [8192, 512] f32
    pos = (Z[:BATCH] * Z[BATCH:]).sum(axis=1)
    posf = np.concatenate([pos, pos]) / TEMP             # [8192]
    Zq = (Z * SCALE).astype(fp8)                         # [8192, 512] fp8
    Zqf = Zq.astype(np.float32) / SCALE
    diag = np.exp((Zqf * Zqf).sum(axis=1) / TEMP)        # [8192]
    ZqT = np.ascontiguousarray(Zq.T)                     # [512, 8192]
    maps = []
    for k in range(NCORES):
        cols = (1024 * k + np.arange(LCOLS)) % B2
        Xc = ZqT[:, cols]                                # [512, 5120]
        xq = np.ascontiguousarray(
            Xc.reshape(4, 128, LCOLS).transpose(1, 0, 2).reshape(128, -1))
        maps.append({"xq": xq})
    return maps, posf, diag


def _run(emb_i, emb_j, trace=False):
    from concourse.bass_utils import run_bass_kernel_spmd
    nc = _get_nc()
    maps, posf, diag = _prep(emb_i, emb_j)
    res = run_bass_kernel_spmd(nc, maps, list(range(NCORES)), trace=trace)
    den = np.zeros(B2, dtype=np.float64)
    for k in range(NCORES):
        rowout = np.asarray(res.results[k]["rowout"], dtype=np.float64)
        colout = np.asarray(res.results[k]["colout"], dtype=np.float64)[0]
        rows = 1024 * k + np.arange(1024)
        den[rows] += rowout.T.reshape(-1)                # [p, j] -> row 128j+p
        g = (1024 * k + CS_LO + np.arange(CSW)) % B2
        den[g] += colout
    den = den - diag
    loss = np.float32(np.mean(np.log(den) - posf))
    return loss, res


def kernel(emb_i, emb_j):
    return _run(emb_i, emb_j, trace=False)[0]


# revision 17
# speedup vs baseline: 2.9743x; 1.0347x over previous
"""Trainium2 Bass kernel for NT-Xent contrastive loss (BATCH=4096, DIM=512, TEMP=0.5).

v2 strategy — exploit the symmetry of the similarity matrix + fp8 DoubleRow:
  - Host: L2-normalize rows of E = concat(emb_i, emb_j) in f32, compute the
    positive-pair dots and the (quantized) diagonal terms exactly, then cast
    z*16 to TRN fp8-e4m3 for the big matmul.
  - The 8192x8192 exp(sim/T) row-sum is split by symmetry: the 64x64 grid of
    128x128 tiles is covered by giving each block-row r the cyclic strip of
    tiles (r, r+c mod 64) for c = 0..32.  Tiles c = 1..31 contribute their
    row-sums to block r's denominators AND their column-sums (via symmetry
    s_ij = s_ji) to the denominators of rows in block r+c.  Tile c = 0 is the
    in-block tile (row-sums only); tile c = 32 appears in both orderings'
    strips, so it is row-sum only as well.  Every ordered pair (i, j) is
    covered exactly once; the self term exp(s_ii/T) is subtracted on host.
  - Core k owns block-rows 8k..8k+7; its rhs is the 5120-column cyclic window
    of z^T starting at column 1024k, so every core runs the IDENTICAL program
    on its own data (SPMD, no collectives).
  - Device per strip: fp8 DoubleRow matmuls (contraction 512 = 2 pairs of 128
    partitions x 2) into [128, 2048] PSUM chunks -> ACT exp (scale = 2/256)
    into a bf16 strip buffer -> DVE row-sum reduce.  Column-sums run as a PE
    tail: all-ones [128,128] weights x exp-strip slices, accumulated across
    strips in shared PSUM (three column phases), DVE-copied and DMA'd out.
  - Host: den = rowsum + colsum - diag; loss = mean(log(den) - pos/TEMP).
"""

import math

import ml_dtypes
import numpy as np

BATCH = 4096
DIM = 512
TEMP = 0.5
B2 = 2 * BATCH            # 8192 rows of the similarity matrix
NCORES = 8
NBLK = B2 // 128          # 64 block-rows
SPB = NBLK // NCORES      # 8 strips (block-rows) per core
NT = 33                   # tiles per strip (c = 0..32)
SW = NT * 128             # 4224 strip width (stream columns per strip)
TOTAL = SPB * SW          # 33792 stream columns per core
LCOLS = 128 * (SPB - 1) + SW   # 5120 local rhs columns per core
CH = 2048                 # main chunk width (PSUM double buffer)
SCALE = 16.0              # fp8 pre-scale on z
ACT_SCALE = (1.0 / TEMP) / (SCALE * SCALE)   # exp(s_hat * ACT_SCALE)
CS_LO = 128               # colsum window (local cols), strips j: [128j+128, 128j+4096)
CS_HI = 128 * (SPB - 1) + 4096   # 4992
CSW = CS_HI - CS_LO       # 4864
PHASES = [(128, 1060), (1060, 1992), (1992, 2924), (2924, 3856),
          (3856, 4736), (4736, 4992)]

_CACHE = {}


def _build():
    import concourse.bacc as bacc
    import concourse.mybir as mybir
    import concourse.tile as tile

    f32 = mybir.dt.float32
    bf16 = mybir.dt.bfloat16
    fp8 = mybir.dt.float8e4
    AF = mybir.ActivationFunctionType
    ALU = mybir.AluOpType
    X = mybir.AxisListType.X
    DR = mybir.MatmulPerfMode.DoubleRow

    nc = bacc.Bacc("TRN2", target_bir_lowering=False, debug=False,
                   num_devices=NCORES)

    xq_d = nc.dram_tensor("xq", [128, 4 * LCOLS], fp8, kind="ExternalInput").ap()
    rowout_d = nc.dram_tensor("rowout", [128, SPB], f32,
                              kind="ExternalOutput").ap()
    colout_d = nc.dram_tensor("colout", [1, CSW], f32,
                              kind="ExternalOutput").ap()

    with tile.TileContext(nc) as tc:
        with (
            tc.tile_pool(name="persist", bufs=1) as P,
            tc.tile_pool(name="scratch", bufs=2) as S,
            tc.tile_pool(name="psum", bufs=2, space="PSUM") as PS,
        ):
            ones = P.tile([128, 128], bf16, name="ones")
            xq = P.tile([128, 4 * LCOLS], fp8, name="xq")
            exps = P.tile([128, TOTAL], bf16, name="exps")
            rowacc = P.tile([128, SPB], f32, name="rowacc")
            colsb = P.tile([1, CSW], f32, name="colsb")

            nc.vector.memset(ones[:], 1.0)
            # HBM loads, first-needed first, balanced over the two DGE queues.
            # The matmul weights are read straight out of xq (cols [0,1024)
            # hold the core's own 1024 rows), so xq is the only input.
            for lo, hi in ((0, 1024), (1024, 2048), (2048, 4224), (4224, LCOLS)):
                for s in (0, 1):
                    sl = slice(LCOLS * s + lo, LCOLS * s + hi)
                    nc.sync.dma_start(xq[:, sl], xq_d[:, sl])
                for s in (2, 3):
                    sl = slice(LCOLS * s + lo, LCOLS * s + hi)
                    nc.gpsimd.dma_start(xq[:, sl], xq_d[:, sl])

            # warm the PE HAM clock gate while the first loads are in flight
            wps = PS.tile([128, 128], f32, tag="mm", name="wps")
            for _ in range(24):
                nc.tensor.matmul(wps[:], ones[:], ones[:], start=True, stop=True)

            xq3 = xq[:].rearrange("p (s c) -> p s c", s=4)

            # stream pieces: cut at the 512 grid (PSUM zero regions) and at
            # strip boundaries
            cuts = sorted(set(
                [512 * m for m in range(TOTAL // 512 + 1)] +
                [SW * j for j in range(SPB + 1)]))
            pieces = list(zip(cuts, cuts[1:]))

            nchunks = (TOTAL + CH - 1) // CH
            reduced = set()
            for ci in range(nchunks):
                c0, c1 = CH * ci, min(TOTAL, CH * (ci + 1))
                cp = [pc for pc in pieces if pc[0] >= c0 and pc[1] <= c1]
                ps = PS.tile([128, c1 - c0], f32, tag="mm", name="ps")
                emit = []
                for j in sorted(set(a // SW for a, b in cp)):
                    for kk in range(2):
                        for (a, b) in cp:
                            if a // SW == j:
                                emit.append((j, kk, a, b))
                regions = {}
                for idx, (j, kk, a, b) in enumerate(emit):
                    regions.setdefault(a // 512, []).append(idx)
                starts = {v[0] for v in regions.values()}
                stops = {v[-1] for v in regions.values()}
                for idx, (j, kk, a, b) in enumerate(emit):
                    lc = a - 4096 * j
                    nc.tensor.matmul(
                        ps[:, a - c0:b - c0],
                        xq3[:, 2 * kk:2 * kk + 2, 128 * j:128 * (j + 1)],
                        xq3[:, 2 * kk:2 * kk + 2, lc:lc + (b - a)],
                        start=(idx in starts), stop=(idx in stops),
                        perf_mode=DR)
                nc.scalar.activation(exps[:, c0:c1], ps[:, 0:c1 - c0],
                                     AF.Exp, scale=ACT_SCALE)
                for j in range(SPB):
                    if j not in reduced and SW * (j + 1) <= c1:
                        # row sums: bf16 pairwise tree folds (DVE 2X_1PORT
                        # needs all-2B operands), then a small f32 reduce
                        with nc.allow_low_precision(
                                reason="pairwise bf16 folds; final add is f32"):
                            a = exps[:, SW * j:SW * (j + 1)]
                            f1 = S.tile([128, 2112], bf16, tag="f1", name="f1")
                            nc.vector.tensor_tensor(
                                f1[:], a[:, 0:2112], a[:, 2112:4224], ALU.add)
                            f2 = S.tile([128, 1056], bf16, tag="f2", name="f2")
                            nc.vector.tensor_tensor(
                                f2[:], f1[:, 0:1056], f1[:, 1056:2112], ALU.add)
                            f3 = S.tile([128, 528], bf16, tag="f3", name="f3")
                            nc.vector.tensor_tensor(
                                f3[:], f2[:, 0:528], f2[:, 528:1056], ALU.add)
                        nc.vector.tensor_reduce(
                            rowacc[:, j:j + 1], f3[:], X, ALU.add)
                        reduced.add(j)

            nc.sync.dma_start(rowout_d[:], rowacc[:])

            # column-sum tail: all-ones weights, accumulate across strips in
            # shared PSUM per column phase, then drain row 0.
            for (pa, pb) in PHASES:
                w = pb - pa
                cps = PS.tile([128, w], f32, tag="mm", name="cps")
                emit = []
                for j in range(SPB):
                    wa = max(pa, 128 * j + 128)
                    wb = min(pb, 128 * j + 4096)
                    if wa >= wb:
                        continue
                    grid = [pa + 512 * g for g in range(1, (w + 511) // 512 + 1)]
                    cpts = [wa] + [g for g in grid if wa < g < wb] + [wb]
                    for a, b in zip(cpts, cpts[1:]):
                        emit.append((j, a, b))
                regions = {}
                for idx, (j, a, b) in enumerate(emit):
                    regions.setdefault((a - pa) // 512, []).append(idx)
                starts = {v[0] for v in regions.values()}
                stops = {v[-1] for v in regions.values()}
                for idx, (j, a, b) in enumerate(emit):
                    so = 4096 * j + a
                    nc.tensor.matmul(
                        cps[:, a - pa:b - pa], ones[:],
                        exps[:, so:so + (b - a)],
                        start=(idx in starts), stop=(idx in stops))
                nc.vector.tensor_copy(colsb[0:1, pa - CS_LO:pb - CS_LO],
                                      cps[0:1, 0:w])
                nc.sync.dma_start(colout_d[0:1, pa - CS_LO:pb - CS_LO],
                                  colsb[0:1, pa - CS_LO:pb - CS_LO])

    nc.compile()
    return nc


def _get_nc():
    if "nc" not in _CACHE:
        _CACHE["nc"] = _build()
    return _CACHE["nc"]


def _prep(emb_i, emb_j):
    fp8 = ml_dtypes.float8_e4m3
    E = np.concatenate([np.asarray(emb_i, dtype=np.float32),
                        np.asarray(emb_j, dtype=np.float32)], axis=0)
    nrm = np.sqrt((E * E).sum(axis=1, keepdims=True))
    Z = E / np.maximum(nrm, 1e-12)                       # [8192, 512] f32
    pos = (Z[:BATCH] * Z[BATCH:]).sum(axis=1)
    posf = np.concatenate([pos, pos]) / TEMP             # [8192]
    Zq = (Z * SCALE).astype(fp8)                         # [8192, 512] fp8
    Zqf = Zq.astype(np.float32) / SCALE
    diag = np.exp((Zqf * Zqf).sum(axis=1) / TEMP)        # [8192]
    ZqT = np.ascontiguousarray(Zq.T)                     # [512, 8192]
    maps = []
    for k in range(NCORES):
        cols = (1024 * k + np.arange(LCOLS)) % B2
        Xc = ZqT[:, cols]                                # [512, 5120]
        xq = np.ascontiguousarray(
            Xc.reshape(4, 128, LCOLS).transpose(1, 0, 2).reshape(128, -1))
        maps.append({"xq": xq})
    return maps, posf, diag


def _run(emb_i, emb_j, trace=False):
    from concourse.bass_utils import run_bass_kernel_spmd
    nc = _get_nc()
    maps, posf, diag = _prep(emb_i, emb_j)
    res = run_bass_kernel_spmd(nc, maps, list(range(NCORES)), trace=trace)
    den = np.zeros(B2, dtype=np.float64)
    for k in range(NCORES):
        rowout = np.asarray(res.results[k]["rowout"], dtype=np.float64)
        colout = np.asarray(res.results[k]["colout"], dtype=np.float64)[0]
        rows = 1024 * k + np.arange(1024)
        den[rows] += rowout.T.reshape(-1)                # [p, j] -> row 128j+p
        g = (1024 * k + CS_LO + np.arange(CSW)) % B2
        den[g] += colout
    den = den - diag
    loss = np.float32(np.mean(np.log(den) - posf))
    return loss, res


def kernel(emb_i, emb_j):
    return _run(emb_i, emb_j, trace=False)[0]
